# revision 38
# baseline (speedup 1.0000x reference)
"""ApproxCompressor Trainium2 kernel (8 NeuronCores, data parallel over batch).

Algorithm: the reference's FFT convolution with the truncated exponential
impulse response h[n] = (1-a) a^n is a one-pole IIR y[t] = a y[t-1] + (1-a) e[t]
minus a tail term a^16384 y[t-16384] that underflows to zero in float32 for
any alpha = sigmoid(randn).  On-device we therefore run an exact recursive
scan instead of an FFT.

v4 layout: 16-bit datapath; the input DMA casts f32->bf16 in flight (SWDGE
casting DMA, the only queue that can cast) and the output DMA casts back,
so HBM stays f32 while DVE elementwise ops run in 2x packed-16-bit mode.
Each example is one [128, 2048] bf16 tile (both channels side by side).

Work split per example (the DVE and GpSimd engines share SBUF ports, so
GpSimd does no elementwise compute at all -- measured: a concurrent Pool
tensor_tensor slows DVE ops by >60%):
  ACT : sq_c = Square(x_c) (bf16, both channels), d = Ln(scale*y + bias)
        (energy scale (1-a)/2 folded into the Ln scale), g = Exp(c*comb-c*W)
  PE  : e = I @ sq0 + I @ sq1 accumulated into PSUM (the channel add),
        plus the [128x128] carry matmul carry = M @ S
  DVE : scan (reads e from PSUM, writes y bf16; fp32 internal state),
        carry STT y[:, :nb*128] += carry * a^(i+1), knee (custom DVE op),
        gain muls g*x_c (bf16 2x)
  POOL: SWDGE casting DMA issue only
  SYNC/VECTOR queues: e0's f32 fast-path input (lands ~2us before the
        SWDGE casting path, shortening the ramp) and the constants

Cross-chunk scan carries are fixed post-hoc: carry[p] (the true initial
state of chunk p) is linear in the per-chunk final values S, carry = M @ S
with M precomputed on host in f64.  One tiny TensorE matmul computes
carry as a per-partition [128,1] PSUM column; a single DVE
scalar_tensor_tensor applies y[:, :nb*128] += carry * a^(i+1) reading
the PSUM scalar operand directly (the decay cutoff is 1e-5 relative,
~30x below the bf16 noise floor of the energy path).

The quadratic-knee gain is refactored into per-partition-scalar ops:
    d    = ln(e^{-thr}*(1-a)/2 * (y_raw + eps'))   (ACT, scale/bias fold)
    comb = (clamp(d,-W,W)+W)^2/(4W) + max(d,W)     (ONE custom-DVE op)
    gain = exp(c*comb - c*W)                       (ACT, scale/bias fold)
which equals exp(c*q(d)) of the reference knee exactly.  The bf16 path
costs ~0.3% relative error, well inside the 2e-2 tolerance.
"""

import numpy as np

N, C, L = 32, 2, 131072
NCORES = 8
NE = N // NCORES          # examples per core
P = 128                   # partitions = chunks per example
F = L // P                # 1024 samples per partition
FC = C * F                # both channels side by side in one tile
BANK = 128                # column granularity of the carry-fix STT
EPS = 1e-5

_CACHE = {}


def _build(nb):
    import concourse.bass as bass
    import concourse.tile as tile
    from concourse import bacc, mybir

    f32 = mybir.dt.float32
    bf16 = mybir.dt.bfloat16
    AF = mybir.ActivationFunctionType
    OP = mybir.AluOpType

    import numpy as _np
    import concourse.dve_ops as _dv
    from concourse.dve_spec import Spec as _Spec, Src0 as _S0, Src1 as _S1, \
        C0 as _C0, C1 as _C1, maxx as _maxx, minn as _minn, sq as _sq
    if "KNEE_COMB_ANT" not in _dv._SUB_OPCODE_FOR_NAME:
        _spec = _Spec(
            body=_sq(_minn(_maxx(_S0, _C0), _C1) + _C1) * _S1 + _maxx(_S0, _C1),
            reference=lambda in0, in1, s0, s1, imm2:
                ((_np.clip(in0, s0, s1) + s1) ** 2) * in1 + _np.maximum(in0, s1),
        )
        _op = _dv.DveOp(
            "KNEE_COMB_ANT", _spec, subdim=False,
            uops_sha={"v3": "f0ac05272d122822", "v4": "43c41ab935255626"})
        _dv._SUB_OPCODE_FOR_NAME[_op.name] = _dv._CUSTOM_DVE_ROW_BASE + len(_dv.OPS)
        _dv.OPS.append(_op)
        _dv.CUSTOM_DVE_SPECS[_op.name] = _spec
    knee_op = next(o for o in _dv.OPS if o.name == "KNEE_COMB_ANT")

    nc = bacc.Bacc("TRN2", target_bir_lowering=False, debug=False, num_devices=1)

    x_h = nc.declare_dram_parameter("x", [NE, C, L], f32, isOutput=False)
    scal_h = nc.declare_dram_parameter("scal", [P, 16 * NE], f32, isOutput=False)
    # eye | mmt0..3 | dec packed into one bf16 tensor -> ONE sync-queue
    # transfer (the HWDGE queue has ~1.5us fixed overhead per transfer;
    # seven small const DMAs made `dec` land at ~20us and stalled the DVE)
    BC = P + NE * P + NE * nb * BANK
    bigc_h = nc.declare_dram_parameter("bigc", [P, BC], bf16, isOutput=False)
    out_h = nc.declare_dram_parameter("out", [NE, C, L], f32, isOutput=True)

    from contextlib import ExitStack

    with tile.TileContext(nc) as tc, ExitStack() as ctx:
        const = ctx.enter_context(tc.tile_pool(name="const", bufs=1))
        work = ctx.enter_context(tc.tile_pool(name="work", bufs=4))
        ypool = ctx.enter_context(tc.tile_pool(name="ypool", bufs=4))
        xpool = ctx.enter_context(tc.tile_pool(name="xpool", bufs=4))
        psum = ctx.enter_context(tc.tile_pool(name="psum", bufs=2, space="PSUM"))

        # warm activation on a memset tile (no DMA dependency): pulls the
        # single ACT table load to the very top of the body.  The memset
        # runs on DVE (idle early) so the pool queue's first instruction
        # is the e0 input DMA issue.
        warm_t = const.tile([P, 1], f32)
        nc.vector.memset(warm_t[:], 0.0)
        nc.scalar.activation(warm_t[:], warm_t[:], AF.Exp, bias=0.0, scale=0.0)

        # all x input streams on the SWDGE ring with the in-flight f32->bf16
        # cast; e0/e1 split per channel (each extra SWDGE transfer costs
        # ~1.7us of serialized queue latency, so no finer)
        xs = []
        for e in range(NE):
            xt = xpool.tile([P, FC], bf16, tag="x", name=f"xe{e}")
            if e <= 1:
                nc.gpsimd.dma_start(
                    xt[:, 0:F], x_h[:][e, 0].rearrange("(p i) -> p i", p=P))
                nc.gpsimd.dma_start(
                    xt[:, F:FC], x_h[:][e, 1].rearrange("(p i) -> p i", p=P))
            else:
                nc.gpsimd.dma_start(
                    xt[:].rearrange("p (c i) -> p c i", c=C),
                    x_h[:][e].rearrange("c (p i) -> p c i", p=P))
            xs.append(xt)

        # constants on the (otherwise idle) sync ring: two transfers total
        scal_t = const.tile([P, 16 * NE], f32)
        nc.sync.dma_start(scal_t[:], scal_h[:])
        bigc_t = const.tile([P, BC], bf16)
        nc.sync.dma_start(bigc_t[:], bigc_h[:])

        def eye_ap():
            return bigc_t[:, 0:P]

        def mmt_ap(e):
            return bigc_t[:, P + e * P : P + (e + 1) * P]

        def dec_ap(e):
            off = P + NE * P + e * nb * BANK
            return bigc_t[:, off : off + nb * BANK]

        def sc(e, j):
            return scal_t[:, 16 * e + j : 16 * e + j + 1]

        HB = F // 2  # psum bank width in f32

        for e in range(NE):
            xt = xs[e]
            x0 = xt[:, 0:F]
            x1 = xt[:, F:FC]

            # energy e[t] = x0^2 + x1^2 (raw; the (1-a)/2 scale is folded
            # into the Ln activation's scale).  Squares on ACT; the channel
            # add runs on the idle PE as two identity matmuls accumulating
            # into PSUM (per 512-col bank) for e1..e3.  e0 adds on DVE
            # instead: the PE pair costs ~2.4us of latency in front of the
            # scan, which would sit on the ramp's critical path.
            sq_t = work.tile([P, FC], bf16, tag="sq")
            y_t = ypool.tile([P, F], bf16, tag="y")
            if e == 0:
                # per-channel squares chase the split input down the ramp
                nc.scalar.activation(sq_t[:, 0:F], x0, AF.Square,
                                     bias=0.0, scale=1.0)
                nc.scalar.activation(sq_t[:, F:FC], x1, AF.Square,
                                     bias=0.0, scale=1.0)
                e_t = work.tile([P, F], bf16, tag="e0add")
                nc.vector.tensor_tensor(e_t[:], sq_t[:, 0:F], sq_t[:, F:FC],
                                        op=OP.add)
                nc.vector.tensor_tensor_scan(
                    y_t[:], sc(e, 1).broadcast_to([P, F]), e_t[:], 0.0,
                    op0=OP.mult, op1=OP.add,
                )
            else:
                # one ACT pass squares both channels (saves the second
                # instruction's fixed overhead)
                nc.scalar.activation(sq_t[:], xt[:], AF.Square,
                                     bias=0.0, scale=1.0)
                e_ps = psum.tile([P, F], f32, tag="eps", bufs=2)
                for h in range(2):
                    cs = slice(h * HB, (h + 1) * HB)
                    nc.tensor.matmul(e_ps[:, cs], eye_ap(), sq_t[:, cs],
                                     start=True, stop=False)
                    nc.tensor.matmul(e_ps[:, cs], eye_ap(), sq_t[:, F + h * HB : F + (h + 1) * HB],
                                     start=False, stop=True)
                nc.vector.tensor_tensor_scan(
                    y_t[:], sc(e, 1).broadcast_to([P, F]), e_ps[:], 0.0,
                    op0=OP.mult, op1=OP.add,
                )

            # carry fix: carryT = S^T @ M^T, then y[:, :nb*128] += carry x decay
            pcar = psum.tile([P, 1], f32, tag="p1", bufs=2)
            nc.tensor.matmul(pcar[:], mmt_ap(e), y_t[:, F - 1 : F],
                             start=True, stop=True)
            nc.vector.scalar_tensor_tensor(
                y_t[:, 0 : nb * BANK], dec_ap(e), pcar[:],
                y_t[:, 0 : nb * BANK], op0=OP.mult, op1=OP.add)

            # knee gain; carry-free upper columns go first so they overlap
            # the carry matmul chain, and the last example additionally
            # splits knee/exp to shorten the drain
            fx = nb * BANK
            d_t = work.tile([P, F], f32, tag="d")
            comb_t = work.tile([P, F], f32, tag="comb")
            g_t = work.tile([P, F], bf16, tag="g")
            split = e in (0, NE - 1) and fx < F
            if split:
                nc.scalar.activation(d_t[:, fx:], y_t[:, fx:], AF.Ln,
                                     bias=sc(e, 3), scale=sc(e, 2))
                nc.scalar.activation(d_t[:, :fx], y_t[:, :fx], AF.Ln,
                                     bias=sc(e, 3), scale=sc(e, 2))
            else:
                nc.scalar.activation(d_t[:], y_t[:], AF.Ln,
                                     bias=sc(e, 3), scale=sc(e, 2))
            if e == NE - 1 and fx < F:
                nc.vector._custom_dve(
                    knee_op, out=comb_t[:, fx:], in0=d_t[:, fx:],
                    in1=sc(e, 10).broadcast_to([P, F - fx]),
                    s0=sc(e, 4), s1=sc(e, 5), imm2=0.0)
                nc.scalar.activation(g_t[:, fx:], comb_t[:, fx:], AF.Exp,
                                     bias=sc(e, 9), scale=sc(e, 6))
                nc.vector._custom_dve(
                    knee_op, out=comb_t[:, :fx], in0=d_t[:, :fx],
                    in1=sc(e, 10).broadcast_to([P, fx]),
                    s0=sc(e, 4), s1=sc(e, 5), imm2=0.0)
                nc.scalar.activation(g_t[:, :fx], comb_t[:, :fx], AF.Exp,
                                     bias=sc(e, 9), scale=sc(e, 6))
            else:
                nc.vector._custom_dve(
                    knee_op, out=comb_t[:], in0=d_t[:],
                    in1=sc(e, 10).broadcast_to([P, F]),
                    s0=sc(e, 4), s1=sc(e, 5), imm2=0.0)
                nc.scalar.activation(g_t[:], comb_t[:], AF.Exp,
                                     bias=sc(e, 9), scale=sc(e, 6))

            # gain application in place on DVE (bf16 2x), then casting DMA
            # out; e3 splits muls/outs at the carry boundary so each region
            # ships the moment its gain lands, cutting the drain
            if e == NE - 1 and fx < F:
                for c in (1, 0):
                    xc = xt[:, c * F : (c + 1) * F]
                    nc.vector.tensor_tensor(xc, g_t[:], xc, op=OP.mult)
                    nc.gpsimd.dma_start(
                        out_h[:][e, c].rearrange("(p i) -> p i", p=P), xc)
            else:
                # one 2048-col TT multiplies both channels by g (read twice
                # via a stride-0 broadcast axis), still in bf16 2x mode
                x3d = xt[:].rearrange("p (c i) -> p c i", c=C)
                g3d = g_t[:].rearrange("p (a i) -> p a i", a=1).broadcast_to(
                    [P, C, F])
                nc.vector.tensor_tensor(x3d, g3d, x3d, op=OP.mult)
                nc.gpsimd.dma_start(
                    out_h[:][e].rearrange("c (p i) -> p c i", p=P),
                    xt[:].rearrange("p (c i) -> p c i", c=C))

    # narrow the ACT table sets so Ln/Exp/Square resolve to the one set that
    # holds all three -> a single table load instead of per-chunk reloads
    import concourse.bacc as bacc_mod

    orig = bacc_mod.get_activation_tables
    strip = {AF.Ln, AF.Exp, AF.Square}

    def patched(arch):
        full = orig(arch)
        return {
            name: (set(fns) if name == "natural_log_exp_and_others"
                   else set(fns) - strip)
            for name, fns in full.items()
        }

    bacc_mod.get_activation_tables = patched
    try:
        nc.compile()
    finally:
        bacc_mod.get_activation_tables = orig
    return nc


def _host_consts(lt, lr, lk, za, nb):
    """Per-core constant tensors from the [NE] parameter vectors (f64 math)."""
    alpha = 1.0 / (1.0 + np.exp(-za))
    thr = lt - 6.0
    r = 1.0 + np.exp(lr)
    c = 1.0 / r - 1.0
    W = np.exp(lk) / 2.0

    cols = np.zeros((NE, 16))
    cols[:, 1] = alpha
    cols[:, 2] = np.exp(-thr) * (1.0 - alpha) / 2.0   # lnscale (energy scale folded)
    cols[:, 3] = EPS * np.exp(-thr)                   # lnbias
    cols[:, 4] = -W
    cols[:, 5] = W
    cols[:, 6] = c                                    # exp scale
    cols[:, 9] = -c * W                               # exp bias
    cols[:, 10] = 1.0 / (4.0 * W)                     # knee-op Src1
    scal = np.tile(cols.reshape(1, NE * 16), (P, 1)).astype(np.float32)

    # carry matrix, transposed for the matmul: mmt[e][q, p] = A^(p-1-q), q < p
    A = alpha**F
    mmt = np.zeros((NE, P, P))
    qs = np.arange(P)
    for e in range(NE):
        for p in range(1, P):
            mmt[e, :p, p] = A[e] ** (p - 1 - qs[:p])
    import ml_dtypes
    mmt = np.concatenate([mmt[e] for e in range(NE)], axis=1)  # [P, NE*P]
    eye = np.eye(P)

    with np.errstate(under="ignore"):
        dec = (alpha[:, None] ** np.arange(1, nb * BANK + 1)[None, :])
    dec = np.tile(dec.reshape(1, NE * nb * BANK), (P, 1))
    bigc = np.concatenate([eye, mmt, dec], axis=1).astype(ml_dtypes.bfloat16)
    return {"scal": scal, "bigc": bigc}


def _pick_nb(za):
    # decay cutoff 2e-4: the dropped correction is <= 2e-4 relative to y,
    # ~20x below the bf16 quantization noise already in the energy path
    alpha_max = float(1.0 / (1.0 + np.exp(-np.max(za))))
    alpha_max = min(max(alpha_max, 1e-6), 1.0 - 1e-9)
    need = np.log(2e-4) / np.log(alpha_max)
    return int(min(max(np.ceil(need / BANK), 1), F // BANK))


def _prep(inputs):
    x = np.ascontiguousarray(np.asarray(inputs["input_signals"], np.float32))
    lt = np.asarray(inputs["log_threshold"], np.float64).reshape(N)
    lr = np.asarray(inputs["log_ratio"], np.float64).reshape(N)
    lk = np.asarray(inputs["log_knee"], np.float64).reshape(N)
    za = np.asarray(inputs["z_alpha_pre"], np.float64).reshape(N)
    nb = _pick_nb(za)
    in_maps = []
    for i in range(NCORES):
        s = slice(i * NE, (i + 1) * NE)
        m = {"x": x[s]}
        m.update(_host_consts(lt[s], lr[s], lk[s], za[s], nb))
        in_maps.append(m)
    return nb, in_maps


def _get_nc(nb):
    if nb not in _CACHE:
        _CACHE[nb] = _build(nb)
    return _CACHE[nb]


def _run(inputs, trace=False):
    from concourse.bass_utils import run_bass_kernel_spmd

    nb, in_maps = _prep(inputs)
    nc = _get_nc(nb)
    res = run_bass_kernel_spmd(nc, in_maps, core_ids=list(range(NCORES)), trace=trace)
    out = np.concatenate([res.results[i]["out"] for i in range(NCORES)], axis=0)
    return out, res


def _probe_ok(out, inputs):
    """Recompute the first partition-chunk (no carry needed there) of two
    examples on the host in f64 and compare -- catches a stale compile-cache
    NEFF or a wedged-device garbage execution."""
    x = np.asarray(inputs["input_signals"], np.float64)
    lt = np.asarray(inputs["log_threshold"], np.float64).reshape(N)
    lr = np.asarray(inputs["log_ratio"], np.float64).reshape(N)
    lk = np.asarray(inputs["log_knee"], np.float64).reshape(N)
    za = np.asarray(inputs["z_alpha_pre"], np.float64).reshape(N)
    for e in (0, N - 1):
        a = 1.0 / (1.0 + np.exp(-za[e]))
        en = (1.0 - a) / 2.0 * (x[e, 0, :F] ** 2 + x[e, 1, :F] ** 2)
        y = np.empty(F)
        s = 0.0
        for i in range(F):
            s = a * s + en[i]
            y[i] = s
        d = np.log(y + EPS) - (lt[e] - 6.0)
        r = 1.0 + np.exp(lr[e])
        c = 1.0 / r - 1.0
        W = np.exp(lk[e]) / 2.0
        u = np.clip(d, -W, W)
        q = (u + W) ** 2 / (4.0 * W) + np.maximum(d - W, 0.0)
        g = np.exp(c * q)
        ref = g[None, :] * x[e, :, :F]
        got = out[e, :, :F].astype(np.float64)
        rel = np.linalg.norm(got - ref) / max(np.linalg.norm(ref), 1e-30)
        if not np.isfinite(rel) or rel > 0.02:
            return False
    return True


def kernel(**inputs):
    out = None
    for attempt in range(3):
        out, _ = _run(inputs, trace=False)
        if _probe_ok(out, inputs):
            return out
        # wrong result: drop compiled state (stale NEFF cache / wedged run)
        # and rebuild from scratch
        import os, shutil

        _CACHE.clear()
        cache_dir = os.environ.get(
            "NEURON_COMPILE_CACHE_URL", "/root/.neuron-compile-cache/"
        )
        if cache_dir and os.path.isdir(cache_dir):
            shutil.rmtree(cache_dir, ignore_errors=True)
            os.makedirs(cache_dir, mode=0o700, exist_ok=True)
    return out


# revision 39
# speedup vs baseline: 1.0728x; 1.0728x over previous
"""ApproxCompressor Trainium2 kernel (8 NeuronCores, data parallel over batch).

Algorithm: the reference's FFT convolution with the truncated exponential
impulse response h[n] = (1-a) a^n is a one-pole IIR y[t] = a y[t-1] + (1-a) e[t]
minus a tail term a^16384 y[t-16384] that underflows to zero in float32 for
any alpha = sigmoid(randn).  On-device we therefore run an exact recursive
scan instead of an FFT.

v4 layout: 16-bit datapath; the input DMA casts f32->bf16 in flight (SWDGE
casting DMA, the only queue that can cast) and the output DMA casts back,
so HBM stays f32 while DVE elementwise ops run in 2x packed-16-bit mode.
Each example is one [128, 2048] bf16 tile (both channels side by side).

Work split per example (the DVE and GpSimd engines share SBUF ports, so
GpSimd does no elementwise compute at all -- measured: a concurrent Pool
tensor_tensor slows DVE ops by >60%):
  ACT : sq_c = Square(x_c) (bf16, both channels), d = Ln(scale*y + bias)
        (energy scale (1-a)/2 folded into the Ln scale), g = Exp(c*comb-c*W)
  PE  : e = I @ sq0 + I @ sq1 accumulated into PSUM (the channel add),
        plus the [128x128] carry matmul carry = M @ S
  DVE : scan (reads e from PSUM, writes y bf16; fp32 internal state),
        carry STT y[:, :nb*128] += carry * a^(i+1), knee (custom DVE op),
        gain muls g*x_c (bf16 2x)
  POOL: SWDGE casting DMA issue only
  SYNC/VECTOR queues: e0's f32 fast-path input (lands ~2us before the
        SWDGE casting path, shortening the ramp) and the constants

Cross-chunk scan carries are fixed post-hoc: carry[p] (the true initial
state of chunk p) is linear in the per-chunk final values S, carry = M @ S
with M precomputed on host in f64.  One tiny TensorE matmul computes
carry as a per-partition [128,1] PSUM column; a single DVE
scalar_tensor_tensor applies y[:, :nb*128] += carry * a^(i+1) reading
the PSUM scalar operand directly (the decay cutoff is 1e-5 relative,
~30x below the bf16 noise floor of the energy path).

The quadratic-knee gain is refactored into per-partition-scalar ops:
    d    = ln(e^{-thr}*(1-a)/2 * (y_raw + eps'))   (ACT, scale/bias fold)
    comb = (clamp(d,-W,W)+W)^2/(4W) + max(d,W)     (ONE custom-DVE op)
    gain = exp(c*comb - c*W)                       (ACT, scale/bias fold)
which equals exp(c*q(d)) of the reference knee exactly.  The bf16 path
costs ~0.3% relative error, well inside the 2e-2 tolerance.
"""

import numpy as np

N, C, L = 32, 2, 131072
NCORES = 8
NE = N // NCORES          # examples per core
P = 128                   # partitions = chunks per example
F = L // P                # 1024 samples per partition
FC = C * F                # both channels side by side in one tile
BANK = 128                # column granularity of the carry-fix STT
EPS = 1e-5

_CACHE = {}


def _build(nb):
    import concourse.bass as bass
    import concourse.tile as tile
    from concourse import bacc, mybir

    f32 = mybir.dt.float32
    bf16 = mybir.dt.bfloat16
    AF = mybir.ActivationFunctionType
    OP = mybir.AluOpType

    import numpy as _np
    import concourse.dve_ops as _dv
    from concourse.dve_spec import Spec as _Spec, Src0 as _S0, Src1 as _S1, \
        C0 as _C0, C1 as _C1, maxx as _maxx, minn as _minn, sq as _sq
    if "KNEE_COMB_ANT" not in _dv._SUB_OPCODE_FOR_NAME:
        _spec = _Spec(
            body=_sq(_minn(_maxx(_S0, _C0), _C1) + _C1) * _S1 + _maxx(_S0, _C1),
            reference=lambda in0, in1, s0, s1, imm2:
                ((_np.clip(in0, s0, s1) + s1) ** 2) * in1 + _np.maximum(in0, s1),
        )
        _op = _dv.DveOp(
            "KNEE_COMB_ANT", _spec, subdim=False,
            uops_sha={"v3": "f0ac05272d122822", "v4": "43c41ab935255626"})
        _dv._SUB_OPCODE_FOR_NAME[_op.name] = _dv._CUSTOM_DVE_ROW_BASE + len(_dv.OPS)
        _dv.OPS.append(_op)
        _dv.CUSTOM_DVE_SPECS[_op.name] = _spec
    knee_op = next(o for o in _dv.OPS if o.name == "KNEE_COMB_ANT")

    nc = bacc.Bacc("TRN2", target_bir_lowering=False, debug=False, num_devices=1)

    x_h = nc.declare_dram_parameter("x", [NE, C, L], f32, isOutput=False)
    scal_h = nc.declare_dram_parameter("scal", [P, 16 * NE], f32, isOutput=False)
    # eye | mmt0..3 | dec packed into one bf16 tensor -> ONE sync-queue
    # transfer (the HWDGE queue has ~1.5us fixed overhead per transfer;
    # seven small const DMAs made `dec` land at ~20us and stalled the DVE)
    BC = P + NE * P + NE * nb * BANK
    bigc_h = nc.declare_dram_parameter("bigc", [P, BC], bf16, isOutput=False)
    out_h = nc.declare_dram_parameter("out", [NE, C, L], f32, isOutput=True)

    from contextlib import ExitStack

    with tile.TileContext(nc) as tc, ExitStack() as ctx:
        const = ctx.enter_context(tc.tile_pool(name="const", bufs=1))
        work = ctx.enter_context(tc.tile_pool(name="work", bufs=4))
        ypool = ctx.enter_context(tc.tile_pool(name="ypool", bufs=4))
        xpool = ctx.enter_context(tc.tile_pool(name="xpool", bufs=4))
        psum = ctx.enter_context(tc.tile_pool(name="psum", bufs=2, space="PSUM"))

        # warm activation on a memset tile (no DMA dependency): pulls the
        # single ACT table load to the very top of the body.  The memset
        # runs on DVE (idle early) so the pool queue's first instruction
        # is the e0 input DMA issue.
        warm_t = const.tile([P, 1], f32)
        nc.vector.memset(warm_t[:], 0.0)
        nc.scalar.activation(warm_t[:], warm_t[:], AF.Exp, bias=0.0, scale=0.0)

        # all x input streams on the SWDGE ring with the in-flight f32->bf16
        # cast; e0/e1 split per channel (each extra SWDGE transfer costs
        # ~1.7us of serialized queue latency, so no finer)
        xs = []
        for e in range(NE):
            xt = xpool.tile([P, FC], bf16, tag="x", name=f"xe{e}")
            if e <= 1:
                nc.gpsimd.dma_start(
                    xt[:, 0:F], x_h[:][e, 0].rearrange("(p i) -> p i", p=P))
                nc.gpsimd.dma_start(
                    xt[:, F:FC], x_h[:][e, 1].rearrange("(p i) -> p i", p=P))
            else:
                nc.gpsimd.dma_start(
                    xt[:].rearrange("p (c i) -> p c i", c=C),
                    x_h[:][e].rearrange("c (p i) -> p c i", p=P))
            xs.append(xt)

        # constants on the (otherwise idle) sync ring: two transfers total
        scal_t = const.tile([P, 16 * NE], f32)
        nc.sync.dma_start(scal_t[:], scal_h[:])
        bigc_t = const.tile([P, BC], bf16)
        nc.sync.dma_start(bigc_t[:], bigc_h[:])

        def eye_ap():
            return bigc_t[:, 0:P]

        def mmt_ap(e):
            return bigc_t[:, P + e * P : P + (e + 1) * P]

        def dec_ap(e):
            off = P + NE * P + e * nb * BANK
            return bigc_t[:, off : off + nb * BANK]

        def sc(e, j):
            return scal_t[:, 16 * e + j : 16 * e + j + 1]

        HB = F // 2  # psum bank width in f32

        for e in range(NE):
            xt = xs[e]
            x0 = xt[:, 0:F]
            x1 = xt[:, F:FC]

            # energy e[t] = x0^2 + x1^2 (raw; the (1-a)/2 scale is folded
            # into the Ln activation's scale).  Squares on ACT; the channel
            # add runs on the idle PE as two identity matmuls accumulating
            # into PSUM (per 512-col bank) for e1..e3.  e0 adds on DVE
            # instead: the PE pair costs ~2.4us of latency in front of the
            # scan, which would sit on the ramp's critical path.
            sq_t = work.tile([P, FC], bf16, tag="sq")
            y_t = ypool.tile([P, F], bf16, tag="y")
            if e == 0:
                # per-channel squares chase the split input down the ramp
                nc.scalar.activation(sq_t[:, 0:F], x0, AF.Square,
                                     bias=0.0, scale=1.0)
                nc.scalar.activation(sq_t[:, F:FC], x1, AF.Square,
                                     bias=0.0, scale=1.0)
                e_t = work.tile([P, F], bf16, tag="e0add")
                nc.vector.tensor_tensor(e_t[:], sq_t[:, 0:F], sq_t[:, F:FC],
                                        op=OP.add)
                nc.vector.tensor_tensor_scan(
                    y_t[:], sc(e, 1).broadcast_to([P, F]), e_t[:], 0.0,
                    op0=OP.mult, op1=OP.add,
                )
            else:
                # one ACT pass squares both channels (saves the second
                # instruction's fixed overhead)
                nc.scalar.activation(sq_t[:], xt[:], AF.Square,
                                     bias=0.0, scale=1.0)
                e_ps = psum.tile([P, F], f32, tag="eps", bufs=2)
                for h in range(2):
                    cs = slice(h * HB, (h + 1) * HB)
                    nc.tensor.matmul(e_ps[:, cs], eye_ap(), sq_t[:, cs],
                                     start=True, stop=False)
                    nc.tensor.matmul(e_ps[:, cs], eye_ap(), sq_t[:, F + h * HB : F + (h + 1) * HB],
                                     start=False, stop=True)
                nc.vector.tensor_tensor_scan(
                    y_t[:], sc(e, 1).broadcast_to([P, F]), e_ps[:], 0.0,
                    op0=OP.mult, op1=OP.add,
                )

            # carry fix: carryT = S^T @ M^T, then y[:, :nb*128] += carry x decay
            pcar = psum.tile([P, 1], f32, tag="p1", bufs=2)
            nc.tensor.matmul(pcar[:], mmt_ap(e), y_t[:, F - 1 : F],
                             start=True, stop=True)
            nc.vector.scalar_tensor_tensor(
                y_t[:, 0 : nb * BANK], dec_ap(e), pcar[:],
                y_t[:, 0 : nb * BANK], op0=OP.mult, op1=OP.add)

            # knee gain; carry-free upper columns go first so they overlap
            # the carry matmul chain, and the last example additionally
            # splits knee/exp to shorten the drain
            fx = nb * BANK
            d_t = work.tile([P, F], f32, tag="d")
            comb_t = work.tile([P, F], f32, tag="comb")
            g_t = work.tile([P, F], bf16, tag="g")
            split = e in (0, NE - 1) and fx < F
            if split:
                nc.scalar.activation(d_t[:, fx:], y_t[:, fx:], AF.Ln,
                                     bias=sc(e, 3), scale=sc(e, 2))
                nc.scalar.activation(d_t[:, :fx], y_t[:, :fx], AF.Ln,
                                     bias=sc(e, 3), scale=sc(e, 2))
            else:
                nc.scalar.activation(d_t[:], y_t[:], AF.Ln,
                                     bias=sc(e, 3), scale=sc(e, 2))
            if e == NE - 1 and fx < F:
                nc.vector._custom_dve(
                    knee_op, out=comb_t[:, fx:], in0=d_t[:, fx:],
                    in1=sc(e, 10).broadcast_to([P, F - fx]),
                    s0=sc(e, 4), s1=sc(e, 5), imm2=0.0)
                nc.scalar.activation(g_t[:, fx:], comb_t[:, fx:], AF.Exp,
                                     bias=sc(e, 9), scale=sc(e, 6))
                nc.vector._custom_dve(
                    knee_op, out=comb_t[:, :fx], in0=d_t[:, :fx],
                    in1=sc(e, 10).broadcast_to([P, fx]),
                    s0=sc(e, 4), s1=sc(e, 5), imm2=0.0)
                nc.scalar.activation(g_t[:, :fx], comb_t[:, :fx], AF.Exp,
                                     bias=sc(e, 9), scale=sc(e, 6))
            else:
                nc.vector._custom_dve(
                    knee_op, out=comb_t[:], in0=d_t[:],
                    in1=sc(e, 10).broadcast_to([P, F]),
                    s0=sc(e, 4), s1=sc(e, 5), imm2=0.0)
                nc.scalar.activation(g_t[:], comb_t[:], AF.Exp,
                                     bias=sc(e, 9), scale=sc(e, 6))

            # gain application in place on DVE (bf16 2x), then casting DMA
            # out; e3 splits muls/outs at the carry boundary so each region
            # ships the moment its gain lands, cutting the drain
            if e == NE - 1 and fx < F:
                for c in (1, 0):
                    xc = xt[:, c * F + fx : (c + 1) * F]
                    nc.vector.tensor_tensor(xc, g_t[:, fx:], xc, op=OP.mult)
                    nc.gpsimd.dma_start(
                        out_h[:][e, c].rearrange("(p i) -> p i", p=P)[:, fx:],
                        xc)
                # the very last pieces (the lo/carry region) are tiny: mul
                # into f32 and ship on the idle sync HWDGE queue, skipping
                # the serialized SWDGE generator at the tail
                xlo = work.tile([P, C * fx], f32, tag="xlo")
                for c in (1, 0):
                    xc = xlo[:, c * fx : (c + 1) * fx]
                    nc.vector.tensor_tensor(
                        xc, g_t[:, :fx], xt[:, c * F : c * F + fx], op=OP.mult)
                    nc.sync.dma_start(
                        out_h[:][e, c].rearrange("(p i) -> p i", p=P)[:, :fx],
                        xc)
            else:
                # one 2048-col TT multiplies both channels by g (read twice
                # via a stride-0 broadcast axis), still in bf16 2x mode
                x3d = xt[:].rearrange("p (c i) -> p c i", c=C)
                g3d = g_t[:].rearrange("p (a i) -> p a i", a=1).broadcast_to(
                    [P, C, F])
                nc.vector.tensor_tensor(x3d, g3d, x3d, op=OP.mult)
                nc.gpsimd.dma_start(
                    out_h[:][e].rearrange("c (p i) -> p c i", p=P),
                    xt[:].rearrange("p (c i) -> p c i", c=C))

    # narrow the ACT table sets so Ln/Exp/Square resolve to the one set that
    # holds all three -> a single table load instead of per-chunk reloads
    import concourse.bacc as bacc_mod

    orig = bacc_mod.get_activation_tables
    strip = {AF.Ln, AF.Exp, AF.Square}

    def patched(arch):
        full = orig(arch)
        return {
            name: (set(fns) if name == "natural_log_exp_and_others"
                   else set(fns) - strip)
            for name, fns in full.items()
        }

    bacc_mod.get_activation_tables = patched
    try:
        nc.compile()
    finally:
        bacc_mod.get_activation_tables = orig
    return nc


def _host_consts(lt, lr, lk, za, nb):
    """Per-core constant tensors from the [NE] parameter vectors (f64 math)."""
    alpha = 1.0 / (1.0 + np.exp(-za))
    thr = lt - 6.0
    r = 1.0 + np.exp(lr)
    c = 1.0 / r - 1.0
    W = np.exp(lk) / 2.0

    cols = np.zeros((NE, 16))
    cols[:, 1] = alpha
    cols[:, 2] = np.exp(-thr) * (1.0 - alpha) / 2.0   # lnscale (energy scale folded)
    cols[:, 3] = EPS * np.exp(-thr)                   # lnbias
    cols[:, 4] = -W
    cols[:, 5] = W
    cols[:, 6] = c                                    # exp scale
    cols[:, 9] = -c * W                               # exp bias
    cols[:, 10] = 1.0 / (4.0 * W)                     # knee-op Src1
    scal = np.tile(cols.reshape(1, NE * 16), (P, 1)).astype(np.float32)

    # carry matrix, transposed for the matmul: mmt[e][q, p] = A^(p-1-q), q < p
    A = alpha**F
    mmt = np.zeros((NE, P, P))
    qs = np.arange(P)
    for e in range(NE):
        for p in range(1, P):
            mmt[e, :p, p] = A[e] ** (p - 1 - qs[:p])
    import ml_dtypes
    mmt = np.concatenate([mmt[e] for e in range(NE)], axis=1)  # [P, NE*P]
    eye = np.eye(P)

    with np.errstate(under="ignore"):
        dec = (alpha[:, None] ** np.arange(1, nb * BANK + 1)[None, :])
    dec = np.tile(dec.reshape(1, NE * nb * BANK), (P, 1))
    bigc = np.concatenate([eye, mmt, dec], axis=1).astype(ml_dtypes.bfloat16)
    return {"scal": scal, "bigc": bigc}


def _pick_nb(za):
    # decay cutoff 2e-4: the dropped correction is <= 2e-4 relative to y,
    # ~20x below the bf16 quantization noise already in the energy path
    alpha_max = float(1.0 / (1.0 + np.exp(-np.max(za))))
    alpha_max = min(max(alpha_max, 1e-6), 1.0 - 1e-9)
    need = np.log(2e-4) / np.log(alpha_max)
    return int(min(max(np.ceil(need / BANK), 1), F // BANK))


def _prep(inputs):
    x = np.ascontiguousarray(np.asarray(inputs["input_signals"], np.float32))
    lt = np.asarray(inputs["log_threshold"], np.float64).reshape(N)
    lr = np.asarray(inputs["log_ratio"], np.float64).reshape(N)
    lk = np.asarray(inputs["log_knee"], np.float64).reshape(N)
    za = np.asarray(inputs["z_alpha_pre"], np.float64).reshape(N)
    nb = _pick_nb(za)
    in_maps = []
    for i in range(NCORES):
        s = slice(i * NE, (i + 1) * NE)
        m = {"x": x[s]}
        m.update(_host_consts(lt[s], lr[s], lk[s], za[s], nb))
        in_maps.append(m)
    return nb, in_maps


def _get_nc(nb):
    if nb not in _CACHE:
        _CACHE[nb] = _build(nb)
    return _CACHE[nb]


def _run(inputs, trace=False):
    from concourse.bass_utils import run_bass_kernel_spmd

    nb, in_maps = _prep(inputs)
    nc = _get_nc(nb)
    res = run_bass_kernel_spmd(nc, in_maps, core_ids=list(range(NCORES)), trace=trace)
    out = np.concatenate([res.results[i]["out"] for i in range(NCORES)], axis=0)
    return out, res


def _probe_ok(out, inputs):
    """Recompute the first partition-chunk (no carry needed there) of two
    examples on the host in f64 and compare -- catches a stale compile-cache
    NEFF or a wedged-device garbage execution."""
    x = np.asarray(inputs["input_signals"], np.float64)
    lt = np.asarray(inputs["log_threshold"], np.float64).reshape(N)
    lr = np.asarray(inputs["log_ratio"], np.float64).reshape(N)
    lk = np.asarray(inputs["log_knee"], np.float64).reshape(N)
    za = np.asarray(inputs["z_alpha_pre"], np.float64).reshape(N)
    for e in (0, N - 1):
        a = 1.0 / (1.0 + np.exp(-za[e]))
        en = (1.0 - a) / 2.0 * (x[e, 0, :F] ** 2 + x[e, 1, :F] ** 2)
        y = np.empty(F)
        s = 0.0
        for i in range(F):
            s = a * s + en[i]
            y[i] = s
        d = np.log(y + EPS) - (lt[e] - 6.0)
        r = 1.0 + np.exp(lr[e])
        c = 1.0 / r - 1.0
        W = np.exp(lk[e]) / 2.0
        u = np.clip(d, -W, W)
        q = (u + W) ** 2 / (4.0 * W) + np.maximum(d - W, 0.0)
        g = np.exp(c * q)
        ref = g[None, :] * x[e, :, :F]
        got = out[e, :, :F].astype(np.float64)
        rel = np.linalg.norm(got - ref) / max(np.linalg.norm(ref), 1e-30)
        if not np.isfinite(rel) or rel > 0.02:
            return False
    return True


def kernel(**inputs):
    out = None
    for attempt in range(3):
        out, _ = _run(inputs, trace=False)
        if _probe_ok(out, inputs):
            return out
        # wrong result: drop compiled state (stale NEFF cache / wedged run)
        # and rebuild from scratch
        import os, shutil

        _CACHE.clear()
        cache_dir = os.environ.get(
            "NEURON_COMPILE_CACHE_URL", "/root/.neuron-compile-cache/"
        )
        if cache_dir and os.path.isdir(cache_dir):
            shutil.rmtree(cache_dir, ignore_errors=True)
            os.makedirs(cache_dir, mode=0o700, exist_ok=True)
    return out
